# revision 1
# baseline (speedup 1.0000x reference)
"""Embedding-lookup v5: bf16 end-to-end on device, f32 upcast on host.

Same proven pipeline as the baseline (4-queue SWDGE non-transpose
gather HBM->SBUF, big contiguous SBUF->HBM writes), but the table and
the device output are bf16: 16.7 MB gather-read + 16.7 MB write per
core (vs 33.5 + 33.5).  W2 = W + b folded host-side then rounded once
to bf16 (rel err ~4e-3, tolerance 2e-2); kernel() upcasts the device
output to f32 on the host.
"""

from contextlib import ExitStack

import numpy as np

import concourse.mybir as mybir
from concourse import bacc, bass_utils, library_config
from concourse._compat import get_trn_type

B, T, D = 8192, 64, 128
NUM_DAYS = 365
N_CORES = 8
ROWS_PER_CORE = B // N_CORES            # 1024
N_IDX = ROWS_PER_CORE * T               # 65536 indices per core

G = 1024                                # indices per dma_gather call
NCALLS = N_IDX // G                     # 64
NBLK = G // 128                         # 8 rows per partition per call
G16 = G // 16
NBUF = 8                                # gather slot ring
NQUEUES = 4

_cache = {}


def _build_bass(reps=1):
    nc = bacc.Bacc(get_trn_type() or "TRN2", num_swdge_queues=NQUEUES)

    idx_l = nc.dram_tensor("idx_l", [NCALLS, 128, G16], mybir.dt.int16,
                           kind="ExternalInput")
    w = nc.dram_tensor("w", [NUM_DAYS, D], mybir.dt.bfloat16,
                       kind="ExternalInput")
    out = nc.dram_tensor("out", [N_IDX, D], mybir.dt.bfloat16,
                         kind="ExternalOutput")

    with ExitStack() as ctx:
        idx_sb = ctx.enter_context(
            nc.sbuf_tensor("idx_sb", [128, NCALLS, G16], mybir.dt.int16))
        g_sb = ctx.enter_context(
            nc.sbuf_tensor("g_sb", [128, NBUF, NBLK, D], mybir.dt.bfloat16))
        sem_idx = ctx.enter_context(nc.semaphore(name="sem_idx"))
        sem_g = [ctx.enter_context(nc.semaphore(name=f"sem_g{i}"))
                 for i in range(NBUF)]
        sem_out = [ctx.enter_context(nc.semaphore(name=f"sem_out{i}"))
                   for i in range(NBUF)]
        block = ctx.enter_context(nc.Block())

        total = reps * NCALLS

        @block.sync
        def _(sync):
            sync.dma_start(idx_sb[:],
                           idx_l[:].rearrange("t p g -> p t g")
                           ).then_inc(sem_idx, 16)
            for t in range(total):
                s, k = t % NBUF, t // NBUF
                sync.wait_ge(sem_g[s], 16 * (k + 1))
                tc = t % NCALLS
                out_ap = out[tc * G:(tc + 1) * G].rearrange(
                    "(p blk) d -> p blk d", p=128)
                sync.dma_start(out_ap, g_sb[:, s]).then_inc(sem_out[s], 16)
            for s in range(NBUF):
                n = total // NBUF + (total % NBUF > s)
                if n:
                    sync.wait_ge(sem_out[s], 16 * n)

        @block.gpsimd
        def _(gpsimd):
            gpsimd.load_library(library_config.mlp)
            gpsimd.wait_ge(sem_idx, 16)
            for t in range(total):
                s, k = t % NBUF, t // NBUF
                if t >= NBUF:
                    gpsimd.wait_ge(sem_out[s], 16 * k)
                gpsimd.dma_gather(
                    g_sb[:, s], w[:, :], idx_sb[:, t % NCALLS],
                    num_idxs=G, num_idxs_reg=G, elem_size=D,
                    queue_num=t % NQUEUES,
                    single_packet=False,
                ).then_inc(sem_g[s], 16)

    nc.compile()
    return nc


def _prep_idx(idx_core: np.ndarray) -> np.ndarray:
    """[N_IDX] int -> [NCALLS, 128, G16] int16 in dma_gather layout."""
    idx3 = idx_core.reshape(NCALLS, 128, NBLK).astype(np.int16)
    fed = idx3.transpose(0, 2, 1).reshape(NCALLS, G)
    wrap = fed.reshape(NCALLS, G16, 16).transpose(0, 2, 1)
    return np.ascontiguousarray(np.tile(wrap, (1, 8, 1)))


def _make_in_maps(batch_positions, W, b):
    import ml_dtypes
    w2 = (np.asarray(W, dtype=np.float32)
          + np.asarray(b, dtype=np.float32)[None, :])
    w2bf = np.ascontiguousarray(w2.astype(ml_dtypes.bfloat16))
    idx = np.asarray(batch_positions).reshape(B, T)
    in_maps = []
    for c in range(N_CORES):
        idx_core = idx[c * ROWS_PER_CORE:(c + 1) * ROWS_PER_CORE].reshape(-1)
        in_maps.append({"idx_l": _prep_idx(idx_core), "w": w2bf})
    return in_maps


def _run(batch_positions, W, b, trace=False):
    if "nc" not in _cache:
        _cache["nc"] = _build_bass()
    nc = _cache["nc"]
    in_maps = _make_in_maps(batch_positions, W, b)
    res = bass_utils.run_bass_kernel_spmd(
        nc, in_maps, core_ids=list(range(N_CORES)), trace=trace)
    out = np.empty((B, T, D), dtype=np.float32)
    for c in range(N_CORES):
        out[c * ROWS_PER_CORE:(c + 1) * ROWS_PER_CORE] = (
            np.asarray(res.results[c]["out"]).astype(np.float32)
            .reshape(ROWS_PER_CORE, T, D))
    return out, res


def kernel(**inputs) -> np.ndarray:
    out, _ = _run(inputs["batch_positions"], inputs["W"], inputs["b"])
    return out



# revision 5
# speedup vs baseline: 1.1078x; 1.1078x over previous
"""Embedding-lookup: bf16 end-to-end on device, f32 upcast on host.

Pipeline per core: SWDGE non-transpose dma_gather (HBM table -> SBUF)
feeding big contiguous SBUF->HBM writes.  W2 = W + b folded host-side,
rounded once to bf16 (rel err ~4e-3, tolerance 2e-2); kernel() upcasts
the device output to f32 on the host.
"""

from contextlib import ExitStack

import numpy as np

import concourse.mybir as mybir
from concourse import bacc, bass_utils, library_config
from concourse._compat import get_trn_type

B, T, D = 8192, 64, 128
NUM_DAYS = 365
N_CORES = 8
ROWS_PER_CORE = B // N_CORES            # 1024
N_IDX = ROWS_PER_CORE * T               # 65536 indices per core

# default variant config (overridable per _build_bass call for experiments)
G = 4096                                # indices per dma_gather call
NBUF = 8                                # gather slot ring
NQUEUES = 4
SINGLE_PACKET = False

_cache = {}


def _build_bass(reps=1, g=None, nbuf=None, nqueues=None, single_packet=None,
                wg=None):
    g = G if g is None else g
    nbuf = NBUF if nbuf is None else nbuf
    nqueues = NQUEUES if nqueues is None else nqueues
    single_packet = SINGLE_PACKET if single_packet is None else single_packet
    wg = g if wg is None else wg                 # write granularity (rows)
    assert wg % g == 0
    sub = wg // g                                # gather calls per write slot

    ncalls = N_IDX // g
    nwrites = N_IDX // wg
    nblk = g // 128
    wblk = wg // 128
    g16 = g // 16

    nc = bacc.Bacc(get_trn_type() or "TRN2", num_swdge_queues=nqueues)

    idx_l = nc.dram_tensor("idx_l", [ncalls, 128, g16], mybir.dt.int16,
                           kind="ExternalInput")
    w = nc.dram_tensor("w", [NUM_DAYS, D], mybir.dt.bfloat16,
                       kind="ExternalInput")
    out = nc.dram_tensor("out", [N_IDX, D], mybir.dt.bfloat16,
                         kind="ExternalOutput")

    with ExitStack() as ctx:
        idx_sb = ctx.enter_context(
            nc.sbuf_tensor("idx_sb", [128, ncalls, g16], mybir.dt.int16))
        g_sb = ctx.enter_context(
            nc.sbuf_tensor("g_sb", [128, nbuf, wblk, D], mybir.dt.bfloat16))
        sem_idx = ctx.enter_context(nc.semaphore(name="sem_idx"))
        sem_g = [ctx.enter_context(nc.semaphore(name=f"sem_g{i}"))
                 for i in range(nbuf)]
        sem_out = [ctx.enter_context(nc.semaphore(name=f"sem_out{i}"))
                   for i in range(nbuf)]
        block = ctx.enter_context(nc.Block())

        total = reps * nwrites                   # write slots over all reps

        @block.sync
        def _(sync):
            sync.dma_start(idx_sb[:],
                           idx_l[:].rearrange("t p g -> p t g")
                           ).then_inc(sem_idx, 16)
            for t in range(total):
                s, k = t % nbuf, t // nbuf
                sync.wait_ge(sem_g[s], 16 * sub * (k + 1))
                tc = t % nwrites
                out_ap = out[tc * wg:(tc + 1) * wg].rearrange(
                    "(p blk) d -> p blk d", p=128)
                sync.dma_start(out_ap, g_sb[:, s]).then_inc(sem_out[s], 16)
            for s in range(nbuf):
                n = total // nbuf + (total % nbuf > s)
                if n:
                    sync.wait_ge(sem_out[s], 16 * n)

        @block.gpsimd
        def _(gpsimd):
            gpsimd.load_library(library_config.mlp)
            gpsimd.wait_ge(sem_idx, 16)
            for t in range(total):
                s, k = t % nbuf, t // nbuf
                if t >= nbuf:
                    gpsimd.wait_ge(sem_out[s], 16 * k)
                for j in range(sub):
                    c = (t % nwrites) * sub + j
                    gpsimd.dma_gather(
                        g_sb[:, s, j * nblk:(j + 1) * nblk],
                        w[:, :], idx_sb[:, c],
                        num_idxs=g, num_idxs_reg=g, elem_size=D,
                        queue_num=(t * sub + j) % nqueues,
                        single_packet=single_packet,
                    ).then_inc(sem_g[s], 16)

    nc.compile()
    return nc


def _prep_idx(idx_core: np.ndarray, g=None, wg=None) -> np.ndarray:
    """[N_IDX] int -> [ncalls, 128, g16] int16 in dma_gather layout.

    Gather call c = (tc, j) fills write-slot tc's blk range
    [j*nblk, (j+1)*nblk); the write maps g_sb[p, blk] -> HBM row
    tc*wg + p*wblk + blk, so fed[c][b*128+p] must be the original index
    at position tc*wg + p*wblk + j*nblk + b.
    """
    g = G if g is None else g
    wg = g if wg is None else wg
    sub = wg // g
    ncalls, nblk, g16 = N_IDX // g, g // 128, g // 16
    wblk = wg // 128
    x = idx_core.astype(np.int16).reshape(N_IDX // wg, 128, sub, nblk)
    fed = x.transpose(0, 2, 3, 1).reshape(ncalls, g)
    wrap = fed.reshape(ncalls, g16, 16).transpose(0, 2, 1)
    return np.ascontiguousarray(np.tile(wrap, (1, 8, 1)))


def _make_in_maps(batch_positions, W, b, g=None, wg=None):
    import ml_dtypes
    w2 = (np.asarray(W, dtype=np.float32)
          + np.asarray(b, dtype=np.float32)[None, :])
    w2bf = np.ascontiguousarray(w2.astype(ml_dtypes.bfloat16))
    idx = np.asarray(batch_positions).reshape(B, T)
    in_maps = []
    for c in range(N_CORES):
        idx_core = idx[c * ROWS_PER_CORE:(c + 1) * ROWS_PER_CORE].reshape(-1)
        in_maps.append({"idx_l": _prep_idx(idx_core, g, wg), "w": w2bf})
    return in_maps


def _run(batch_positions, W, b, trace=False):
    if "nc" not in _cache:
        _cache["nc"] = _build_bass()
    nc = _cache["nc"]
    in_maps = _make_in_maps(batch_positions, W, b)
    res = bass_utils.run_bass_kernel_spmd(
        nc, in_maps, core_ids=list(range(N_CORES)), trace=trace)
    out = np.empty((B, T, D), dtype=np.float32)
    for c in range(N_CORES):
        out[c * ROWS_PER_CORE:(c + 1) * ROWS_PER_CORE] = (
            np.asarray(res.results[c]["out"]).astype(np.float32)
            .reshape(ROWS_PER_CORE, T, D))
    return out, res


def kernel(**inputs) -> np.ndarray:
    out, _ = _run(inputs["batch_positions"], inputs["W"], inputs["b"])
    return out
